# revision 4
# baseline (speedup 1.0000x reference)
"""CrossModalityAttention Trainium2 kernel (8 NeuronCores, SPMD, no collectives).

v3 = v2 (fp8 DoubleRow datapath) +
- PE transposes replaced by XBAR DMA transposes (SBUF->SBUF, bf16) with the
  fp8 conversion folded into the existing permute-copy; frees ~64us of PE.
- Software-pipelined attention: scores+exp for s-block-pair u+1 are emitted
  BEFORE the PV matmuls of pair u, so the in-order PE queue never stalls on
  the ACT engine's exp latency (phase B was losing ~50us to that).
- Per-head zinv broadcast via DRAM round trip (write [1,512], read back
  [64,512] with a stride-0 partition AP) instead of a PE ones-matmul; the
  round trip hides under the next head-pair's scores and frees a PSUM bank
  so both PV accumulators double-buffer across head pairs.
- Head-pair finalize (zinv/normalize) deferred until after the next pair's
  first scores are emitted, keeping the PE fed across the boundary.
- Startup: activation DMAs issued before weight DMAs.

Sharding unchanged: core c -> batch c//4, query-row block c%4, all heads,
full K/V computed locally (duplicated), zero cross-core dependencies.
"""
import sys
import numpy as np
import ml_dtypes

for p in ("/root/.axon_site", "/root/.axon_site/_ro/trn_rl_repo",
          "/root/.axon_site/_ro/pypackages", "/opt/trn_rl_repo"):
    if p not in sys.path:
        sys.path.append(p)

import concourse.bass as bass
from concourse import bacc
import concourse.mybir as mybir
import concourse.tile as tile
from concourse.bass_utils import run_bass_kernel_spmd

f32 = mybir.dt.float32
bf16 = mybir.dt.bfloat16
f8 = mybir.dt.float8e4
AF = mybir.ActivationFunctionType
ALU = mybir.AluOpType
PM = mybir.MatmulPerfMode

B, T, S, DIM = 2, 2048, 2048, 1024
HEADS, HEAD_DIM = 16, 64
N_CORES = 8
CORE_IDS = list(range(N_CORES))
EPS = 1e-5
SCALE = HEAD_DIM ** -0.5

QR = 512                  # query rows per core
NQT = QR // 128           # 4 q tiles
NSB = (T + S) // 128      # 32 s-blocks
NU = NSB // 2             # 16 s-block pairs (PV DoubleRow granularity)
VW = HEAD_DIM + 1         # V columns + ones column per head


def _build():
    nc = bacc.Bacc("TRN2", target_bir_lowering=False, debug=False,
                   num_devices=N_CORES)

    XQ = nc.dram_tensor("xq", [QR, DIM], bf16, kind="ExternalInput").ap()
    XB = nc.dram_tensor("xb", [T, DIM], bf16, kind="ExternalInput").ap()
    CB = nc.dram_tensor("cb", [S, DIM], bf16, kind="ExternalInput").ap()
    WQ = nc.dram_tensor("wq", [DIM, DIM], f8, kind="ExternalInput").ap()
    WK = nc.dram_tensor("wk", [DIM, DIM], f8, kind="ExternalInput").ap()
    WV = nc.dram_tensor("wv", [DIM, DIM], f8, kind="ExternalInput").ap()
    WO = nc.dram_tensor("wo", [DIM, DIM], bf16, kind="ExternalInput").ap()
    BQ = nc.dram_tensor("bq", [DIM], f32, kind="ExternalInput").ap()
    BK = nc.dram_tensor("bk", [DIM], f32, kind="ExternalInput").ap()
    BV = nc.dram_tensor("bv", [DIM], f32, kind="ExternalInput").ap()
    RES = nc.dram_tensor("res", [QR, DIM], f32, kind="ExternalInput").ap()

    OUT = nc.dram_tensor("out", [QR, DIM], f32, kind="ExternalOutput").ap()
    zscr = nc.dram_tensor("zscr", [HEADS, 512], f32).ap()

    with tile.TileContext(nc) as tc:
        with (
            tc.tile_pool(name="persist", bufs=1) as per,
            tc.tile_pool(name="wpool", bufs=2) as wp,
        ):
            # ---------------- persistent tiles ----------------
            kt_sb = per.tile([128, 8, T + S], f8, tag="kt")        # K^T concat
            v_sb = per.tile([128, NSB, HEADS * VW], f8, tag="v")   # V | ones
            qt_sb = per.tile([128, 8, QR], f8, tag="qt")           # Q^T
            bq_sb = per.tile([128, 8], f32, tag="bq")
            bk_sb = per.tile([128, 8], f32, tag="bk")
            wo_sb = per.tile([128, 8, DIM], bf16, tag="wo")

            nc.sync.dma_start(out=bq_sb, in_=BQ.rearrange("(a p) -> p a", p=128))
            nc.sync.dma_start(out=bk_sb, in_=BK.rearrange("(a p) -> p a", p=128))
            for h in range(HEADS):  # ones columns for Z rows
                nc.vector.memset(v_sb[:, :, h * VW + HEAD_DIM:(h + 1) * VW], 1.0)

            # ---------------- phase A: LN + transposes + projections --------
            with (
                tc.tile_pool(name="st", bufs=1) as st,
                tc.tile_pool(name="xnp", bufs=5) as xnp,
                tc.tile_pool(name="xntbp", bufs=2) as xntbp,
                tc.tile_pool(name="xntp", bufs=2) as xntp,
                tc.tile_pool(name="psA", bufs=2, space="PSUM") as psA,
            ):
                bvb = st.tile([128, DIM], f32, tag="bvb")
                nc.sync.dma_start(out=bvb, in_=bass.AP(
                    tensor=BV.tensor, offset=0, ap=[[0, 128], [1, DIM]]))
                eps_sb = st.tile([128, 1], f32, tag="eps")
                nc.vector.memset(eps_sb, EPS)

                def ln_tile(SRC, r0):
                    """LN one 128-row tile -> normalized bf16 tile (gamma/beta
                    folded into the weights host-side)."""
                    xt = st.tile([128, DIM], bf16, tag="xt", bufs=3)
                    nc.sync.dma_start(out=xt, in_=SRC[r0:r0 + 128, :])
                    xn = xnp.tile([128, DIM], bf16, tag="xn")
                    sums = st.tile([128, 1], f32, tag="sums", bufs=2)
                    sq = st.tile([128, 1], f32, tag="sq", bufs=2)
                    nc.scalar.activation(out=xn, in_=xt, func=AF.Copy,
                                         accum_out=sums)
                    nc.scalar.activation(out=xn, in_=xt, func=AF.Square,
                                         accum_out=sq)
                    mean = st.tile([128, 1], f32, tag="mean", bufs=2)
                    nc.vector.tensor_scalar(
                        out=mean, in0=sums, scalar1=1.0 / DIM, scalar2=None,
                        op0=ALU.mult)
                    varr = st.tile([128, 1], f32, tag="varr", bufs=2)
                    nc.vector.tensor_tensor(out=varr, in0=sums, in1=mean,
                                            op=ALU.mult)
                    nc.vector.tensor_tensor(out=varr, in0=sq, in1=varr,
                                            op=ALU.subtract)
                    rstd = st.tile([128, 1], f32, tag="rstd", bufs=2)
                    nc.scalar.activation(out=rstd, in_=varr, func=AF.Sqrt,
                                         bias=eps_sb, scale=1.0 / DIM)
                    nc.vector.reciprocal(out=rstd, in_=rstd)
                    nc.vector.tensor_scalar(
                        out=xn, in0=xt, scalar1=mean, scalar2=rstd,
                        op0=ALU.subtract, op1=ALU.mult)
                    return xn

                def transpose_chunk(xn_tiles):
                    """[4 x [128t, 1024c]] -> fp8 xnt [128c, 8ckt, 512t].
                    XBAR DMA transpose (bf16) + fp8 conversion in the
                    block-permuting copy."""
                    xnt_bf = xntbp.tile([128, 4, 8, 128], bf16, tag="xntb")
                    for tt in range(4):
                        nc.sync.dma_start_transpose(out=xnt_bf[:, tt, :, :],
                                                    in_=xn_tiles[tt])
                    xnt = xntp.tile([128, 8, 512], f8, tag="xnt")
                    for ckt in range(8):
                        nc.vector.tensor_copy(xnt[:, ckt, :],
                                              xnt_bf[:, :, ckt, :])
                    return xnt

                def proj_dr(psum, w, xnt, cols):
                    for t2 in range(4):
                        nc.tensor.matmul(
                            psum,
                            lhsT=w[:, 2 * t2:2 * t2 + 2, cols],
                            rhs=xnt[:, 2 * t2:2 * t2 + 2, :],
                            start=(t2 == 0), stop=(t2 == 3),
                            perf_mode=PM.DoubleRow)

                # ---- pass 1: Q projection for this core's 512 rows ----
                xn_tiles = [ln_tile(XQ, tt * 128) for tt in range(NQT)]

                # weight ring: wq -> buf0, wk -> buf1, wv -> buf0 (after
                # q-proj). Issued after the XQ activation DMAs so the first
                # LN isn't queued behind 4MB of weights.
                wq_sb = wp.tile([128, 8, DIM], f8, tag="w")
                wk_sb = wp.tile([128, 8, DIM], f8, tag="w")
                nc.sync.dma_start(out=wq_sb,
                                  in_=WQ.rearrange("(a p) c -> p a c", p=128))
                nc.sync.dma_start(out=wk_sb,
                                  in_=WK.rearrange("(a p) c -> p a c", p=128))

                xnt = transpose_chunk(xn_tiles)
                for chb in range(8):
                    pq = psA.tile([128, 512], f32, tag="kp")
                    proj_dr(pq, wq_sb, xnt, slice(chb * 128, (chb + 1) * 128))
                    nc.vector.tensor_scalar(
                        out=qt_sb[:, chb, :], in0=pq,
                        scalar1=bq_sb[:, chb:chb + 1], scalar2=None,
                        op0=ALU.add)

                wv_sb = wp.tile([128, 8, DIM], f8, tag="w")
                nc.sync.dma_start(out=wv_sb,
                                  in_=WV.rearrange("(a p) c -> p a c", p=128))
                nc.sync.dma_start(out=wo_sb,
                                  in_=WO.rearrange("(a p) c -> p a c", p=128))

                # ---- pass 2: K^T and V for self + context rows ----
                for src_i, SRC in ((0, XB), (1, CB)):
                    for ch in range(4):
                        gch = src_i * 4 + ch
                        xn_tiles = [ln_tile(SRC, (ch * 4 + tt) * 128)
                                    for tt in range(4)]
                        xnt = transpose_chunk(xn_tiles)
                        for chb in range(8):
                            pk = psA.tile([128, 512], f32, tag="kp")
                            proj_dr(pk, wk_sb, xnt,
                                    slice(chb * 128, (chb + 1) * 128))
                            nc.vector.tensor_scalar(
                                out=kt_sb[:, chb, gch * 512:(gch + 1) * 512],
                                in0=pk, scalar1=bk_sb[:, chb:chb + 1],
                                scalar2=None, op0=ALU.add)
                        for tt in range(4):
                            sb_i = gch * 4 + tt
                            for half in range(2):
                                pv = psA.tile([128, 512], f32, tag="vp")
                                for t2 in range(4):
                                    nc.tensor.matmul(
                                        pv,
                                        lhsT=xnt[:, 2 * t2:2 * t2 + 2,
                                                 tt * 128:(tt + 1) * 128],
                                        rhs=wv_sb[:, 2 * t2:2 * t2 + 2,
                                                  half * 512:(half + 1) * 512],
                                        start=(t2 == 0), stop=(t2 == 3),
                                        perf_mode=PM.DoubleRow)
                                dst = v_sb[:, sb_i,
                                           half * 8 * VW:(half * 8 + 8) * VW
                                           ].rearrange("p (h w) -> p h w",
                                                       h=8)[:, :, 0:HEAD_DIM]
                                nc.vector.tensor_tensor(
                                    out=dst,
                                    in0=pv[:].rearrange("p (h d) -> p h d", h=8),
                                    in1=bvb[:, half * 512:(half + 1) * 512
                                            ].rearrange("p (h d) -> p h d", h=8),
                                    op=ALU.add)

            # ---------------- phase B: attention (software-pipelined) -----
            with (
                tc.tile_pool(name="ep", bufs=3) as ep,
                tc.tile_pool(name="zp", bufs=2) as zp,
                tc.tile_pool(name="aotp", bufs=1) as aotp,
                tc.tile_pool(name="psB", bufs=1, space="PSUM") as psB,
            ):
                aot_sb = aotp.tile([128, 8, QR], bf16, tag="aot")

                def emit_scores(hp, u, e_t):
                    for sbp in range(2):
                        sb_i = 2 * u + sbp
                        for h2 in range(2):
                            ps = psB.tile([128, 512], f32, tag=f"sc{h2}",
                                          bufs=2, name=f"ps{h2}")
                            nc.tensor.matmul(
                                ps,
                                lhsT=kt_sb[h2 * 64:(h2 + 1) * 64, hp,
                                           sb_i * 128:(sb_i + 1) * 128],
                                rhs=qt_sb[h2 * 64:(h2 + 1) * 64, hp, :],
                                start=True, stop=True)
                            nc.scalar.activation(out=e_t[h2][:, sbp, :],
                                                 in_=ps, func=AF.Exp,
                                                 scale=SCALE)

                def new_ets():
                    return [ep.tile([128, 2, 512], f8, tag=f"e{h2}",
                                    name=f"et{h2}") for h2 in range(2)]

                def finalize(hp, po):
                    for h2 in range(2):
                        hrow = hp * 2 + h2
                        zi = zp.tile([1, 512], f32, tag="zi")
                        nc.vector.reciprocal(out=zi, in_=po[h2][HEAD_DIM:VW, :])
                        nc.sync.dma_start(out=zscr[hrow:hrow + 1, :], in_=zi)
                        zbs = zp.tile([HEAD_DIM, 512], f32, tag="zbs")
                        row = zscr[hrow:hrow + 1, :]
                        nc.sync.dma_start(out=zbs, in_=bass.AP(
                            tensor=row.tensor, offset=row.offset,
                            ap=[[0, HEAD_DIM]] + list(row.ap[1:])))
                        nc.vector.tensor_tensor(
                            out=aot_sb[h2 * 64:(h2 + 1) * 64, hp, :],
                            in0=po[h2][0:HEAD_DIM, :], in1=zbs,
                            op=ALU.mult)

                pending = None
                for hp in range(8):
                    po = [psB.tile([VW, 512], f32, tag=f"pv{h2}", bufs=2,
                                   name=f"po{h2}") for h2 in range(2)]
                    ets = new_ets()
                    emit_scores(hp, 0, ets)
                    if pending is not None:
                        finalize(*pending)
                    for u in range(NU):
                        cur = ets
                        if u < NU - 1:
                            ets = new_ets()
                            emit_scores(hp, u + 1, ets)
                        for h2 in range(2):
                            h = hp * 2 + h2
                            nc.tensor.matmul(
                                po[h2],
                                lhsT=v_sb[:, 2 * u:2 * u + 2,
                                          h * VW:(h + 1) * VW],
                                rhs=cur[h2],
                                start=(u == 0), stop=(u == NU - 1),
                                perf_mode=PM.DoubleRow)
                    pending = (hp, po)
                finalize(*pending)

            # ---------------- phase C: output projection + residual ------
            with (
                tc.tile_pool(name="op", bufs=2) as op,
                tc.tile_pool(name="psC", bufs=2, space="PSUM") as psC,
            ):
                for tt in range(NQT):
                    re_sb = op.tile([128, DIM], f32, tag="re")
                    nc.sync.dma_start(out=re_sb,
                                      in_=RES[tt * 128:(tt + 1) * 128, :])
                    for half in range(2):
                        pp = psC.tile([128, 512], f32, tag="pp")
                        for chb in range(8):
                            nc.tensor.matmul(
                                pp,
                                lhsT=aot_sb[:, chb, tt * 128:(tt + 1) * 128],
                                rhs=wo_sb[:, chb, half * 512:(half + 1) * 512],
                                start=(chb == 0), stop=(chb == 7))
                        o_sb = op.tile([128, 512], f32, tag="o")
                        nc.vector.tensor_tensor(
                            out=o_sb, in0=pp,
                            in1=re_sb[:, half * 512:(half + 1) * 512],
                            op=ALU.add)
                        nc.sync.dma_start(
                            out=OUT[tt * 128:(tt + 1) * 128,
                                    half * 512:(half + 1) * 512],
                            in_=o_sb)

    nc.compile()
    return nc


_NC = None


def _get_nc():
    global _NC
    if _NC is None:
        _NC = _build()
    return _NC


def make_in_maps(x, context, w_qkv, b_qkv, w_out, b_out, ln_g, ln_b):
    x = np.asarray(x, np.float32)
    context = np.asarray(context, np.float32)
    w_qkv = np.asarray(w_qkv, np.float32)
    b_qkv = np.asarray(b_qkv, np.float32)
    w_out = np.asarray(w_out, np.float32)
    b_out = np.asarray(b_out, np.float32)
    ln_g = np.asarray(ln_g, np.float32)
    ln_b = np.asarray(ln_b, np.float32)

    gw = ln_g[:, None] * w_qkv          # fold LN gamma into W
    bias_full = b_qkv + ln_b @ w_qkv    # fold LN beta into bias
    wq = gw[:, :DIM].astype(ml_dtypes.float8_e4m3)
    wk = gw[:, DIM:2 * DIM].astype(ml_dtypes.float8_e4m3)
    wv = gw[:, 2 * DIM:].astype(ml_dtypes.float8_e4m3)
    wo = w_out.astype(ml_dtypes.bfloat16)
    bq = bias_full[:DIM].astype(np.float32)
    bk = bias_full[DIM:2 * DIM].astype(np.float32)
    bv = bias_full[2 * DIM:].astype(np.float32)

    xb_bf = [x[b].astype(ml_dtypes.bfloat16) for b in range(B)]
    cb_bf = [context[b].astype(ml_dtypes.bfloat16) for b in range(B)]

    in_maps = []
    for c in range(N_CORES):
        b, q = divmod(c, 4)
        rows = slice(q * QR, (q + 1) * QR)
        in_maps.append({
            "xq": xb_bf[b][rows],
            "xb": xb_bf[b], "cb": cb_bf[b],
            "wq": wq, "wk": wk, "wv": wv, "wo": wo,
            "bq": bq, "bk": bk, "bv": bv,
            "res": (x[b, rows, :] + b_out).astype(np.float32),
        })
    return in_maps


def kernel(x, context, w_qkv, b_qkv, w_out, b_out, ln_g, ln_b):
    in_maps = make_in_maps(x, context, w_qkv, b_qkv, w_out, b_out, ln_g, ln_b)
    res = run_bass_kernel_spmd(_get_nc(), in_maps, CORE_IDS)
    out = np.empty((B, T, DIM), np.float32)
    for c in range(N_CORES):
        b, q = divmod(c, 4)
        out[b, q * QR:(q + 1) * QR, :] = res.results[c]["out"]
    return out
